# revision 1
# baseline (speedup 1.0000x reference)
"""HGATConv on 8 trn2 NeuronCores via Bass/Tile.

Math (equivalent to reference, softmax without max-shift — values are small):
  h = x@W + b;  a_n = h@attn_node;  e = exp(a_n)
  stage1: hhat_e[j] = sum_{i: he_i=j} e[n_i]*h[n_i];  q[j] = sum e[n_i]
          S1 = sum_n counts[n]*e[n]
  a_e = (hhat_e @ attn_edge)/S1;  u = exp(a_e)
  S2 = sum_j u[j]*q[j]
  T2[j] = u[j]*hhat_e[j]/S1
  h_n[n] = e[n]/S2 * sum_{i: node_i=n} T2[he_i]

Three SPMD launches (host concatenates shards between launches, off-clock):
  A: per-core node shard -> g table rows [e*h | e | pad] (768B) + e columns
  B: stage-1: dma_gather g rows (sorted by dest hyperedge), Sel-matmul
     segment-sum -> per-core he_part [HSH, 129] (hhat_e | q)
  C: S1, a_e/u/T2/S2 from replicated he_part; stage-2 dma_gather T2 rows
     (sorted by dest node), Sel-matmul segment-sum, scale -> h_n shard.
"""
import os
import sys

sys.path.insert(0, os.path.dirname(os.path.abspath(__file__)))
try:
    import ntff_shim  # noqa: F401  (optional; enables trace under axon)
except Exception:
    pass

import numpy as np
import concourse.bacc as bacc
import concourse.mybir as mybir
import concourse.tile as tile
from concourse.bass_utils import run_bass_kernel_spmd

f32 = mybir.dt.float32
i16 = mybir.dt.int16
P = 128
NC = 8
N, H, M, D = 100000, 20000, 600000, 128
NSH, HSH = N // NC, H // NC          # 12500, 2500
NB1 = (HSH + P - 1) // P             # 20 stage-1 dest blocks/core (last 68)
NB2 = (NSH + P - 1) // P             # 98 stage-2 dest blocks/core (last 84)
NSHP = NB2 * P                       # 12544
CHUNK = 25000                        # stage-1 gather-table chunk rows (int16 idx)
NK = 4
EW = 192                             # g row floats (768B): [e*h(128) | e | 0pad]
GB1 = 2                              # stage-1 blocks per gather call group
GB2 = 4                              # stage-2 blocks per gather call group
HT = (H + P - 1) // P                # 157 he tiles in launch C
HP = HT * P                          # 20096

LAST_EXEC_TIMES = []

_TRACE = bool(os.environ.get("HGAT_TRACE"))


def _run(nc, ins, tag):
    nc.finalize()
    res = run_bass_kernel_spmd(nc, ins, list(range(NC)), trace=_TRACE)
    if _TRACE:
        LAST_EXEC_TIMES.append((tag, res.exec_time_ns, res.mean_exec_time_ns))
    return res.results


def _wrap16(idx):
    """dma_gather index layout: i -> [i%16, i//16], replicated to 128 rows."""
    a = idx.reshape(-1, 16).T
    return np.tile(a, (8, 1)).astype(np.int16)


def _pack_groups(key, ngroups, gidx_val, rel_val, ncores, per_core_groups):
    """Sort incidences by group, pad each group to a core-uniform chunk count.

    Returns per-core (gidx int16 [Mc], rel f32 [Mc], CB dict group->chunks).
    """
    order = np.argsort(key, kind="stable")
    key_s = key[order]
    gv = gidx_val[order]
    rv = rel_val[order]
    counts = np.bincount(key_s, minlength=ngroups * ncores)
    starts = np.zeros(ngroups * ncores + 1, np.int64)
    np.cumsum(counts, out=starts[1:])
    cb = np.maximum(
        1, (counts.reshape(ncores, ngroups) + P - 1) // P
    ).max(axis=0)  # [ngroups] chunks per group, uniform across cores
    out = []
    for c in range(ncores):
        gparts, rparts = [], []
        for g in range(ngroups):
            s, e = starts[c * ngroups + g], starts[c * ngroups + g + 1]
            pad = cb[g] * P - (e - s)
            gparts.append(gv[s:e])
            gparts.append(np.zeros(pad, gv.dtype))
            rparts.append(rv[s:e])
            rparts.append(np.full(pad, -1.0, np.float32))
        out.append((np.concatenate(gparts), np.concatenate(rparts)))
    return out, cb


def _build_launch_a():
    nc = bacc.Bacc("TRN2")
    xT = nc.declare_dram_parameter("xT", [P, NSHP], f32, isOutput=False)
    Wp = nc.declare_dram_parameter("W", [P, D], f32, isOutput=False)
    b_bc = nc.declare_dram_parameter("b_bc", [P, D], f32, isOutput=False)
    an_bc = nc.declare_dram_parameter("an_bc", [P, D], f32, isOutput=False)
    g_sh = nc.declare_dram_parameter("g_sh", [NSHP, EW], f32, isOutput=True)
    exan_sh = nc.declare_dram_parameter("exan_sh", [P, NB2], f32, isOutput=True)

    with tile.TileContext(nc) as tc:
        with (
            tc.tile_pool(name="sbuf", bufs=1) as pool,
            tc.tile_pool(name="work", bufs=4) as wpool,
            tc.tile_pool(name="psum", bufs=4, space="PSUM") as pp,
        ):
            xt = pool.tile([P, NSHP], f32)
            nc.sync.dma_start(out=xt[:], in_=xT[:])
            wt = pool.tile([P, D], f32)
            nc.sync.dma_start(out=wt[:], in_=Wp[:])
            bt = pool.tile([P, D], f32)
            nc.sync.dma_start(out=bt[:], in_=b_bc[:])
            at = pool.tile([P, D], f32)
            nc.sync.dma_start(out=at[:], in_=an_bc[:])
            exan = pool.tile([P, NB2], f32)
            for t in range(NB2):
                ps = pp.tile([P, D], f32, tag="ph", space="PSUM")
                nc.tensor.matmul(
                    out=ps[:], lhsT=xt[:, t * P : (t + 1) * P], rhs=wt[:],
                    start=True, stop=True,
                )
                ht = wpool.tile([P, D], f32, tag="ht")
                nc.vector.tensor_tensor(
                    out=ht[:], in0=ps[:], in1=bt[:], op=mybir.AluOpType.add
                )
                # a_n = sum_free(h*attn)
                tmp = wpool.tile([P, D], f32, tag="tmp")
                acol = wpool.tile([P, 1], f32, tag="acol")
                nc.vector.tensor_tensor(
                    out=tmp[:], in0=ht[:], in1=at[:], op=mybir.AluOpType.mult
                )
                nc.vector.tensor_reduce(
                    out=acol[:], in_=tmp[:], axis=mybir.AxisListType.X,
                    op=mybir.AluOpType.add,
                )
                ecol = wpool.tile([P, 1], f32, tag="ecol")
                nc.scalar.activation(
                    out=ecol[:], in_=acol[:],
                    func=mybir.ActivationFunctionType.Exp,
                )
                gt = wpool.tile([P, EW], f32, tag="gt")
                nc.gpsimd.memset(gt[:, D + 1 :], 0)
                nc.scalar.activation(
                    out=gt[:, 0:D], in_=ht[:],
                    func=mybir.ActivationFunctionType.Copy, scale=ecol[:],
                )
                nc.vector.tensor_copy(out=gt[:, D : D + 1], in_=ecol[:])
                nc.vector.tensor_copy(out=exan[:, t : t + 1], in_=ecol[:])
                nc.sync.dma_start(out=g_sh[t * P : (t + 1) * P, :], in_=gt[:])
            nc.sync.dma_start(out=exan_sh[:], in_=exan[:])
    return nc


MAXCH = 8  # dma_gather cap: 1024 indices per call


def _build_launch_b(cb1, col_of):
    TOT1 = int(cb1.sum())
    cbm = cb1.reshape(NB1, NK)
    nc = bacc.Bacc("TRN2")
    gk = [
        nc.declare_dram_parameter(f"g{k}", [CHUNK, EW], f32, isOutput=False)
        for k in range(NK)
    ]
    idxs = nc.declare_dram_parameter("idxs", [P, TOT1 * 8], i16, isOutput=False)
    herel = nc.declare_dram_parameter("herel", [P, TOT1], f32, isOutput=False)
    iota = nc.declare_dram_parameter("iota", [P, P], f32, isOutput=False)
    he_part = nc.declare_dram_parameter("he_part", [HSH, D + 1], f32, isOutput=True)

    with tile.TileContext(nc) as tc:
        with (
            tc.tile_pool(name="sbuf", bufs=1) as pool,
            tc.tile_pool(name="gpool", bufs=8) as gpool,
            tc.tile_pool(name="work", bufs=4) as wpool,
            tc.tile_pool(name="psum", bufs=4, space="PSUM") as pp,
        ):
            idx_t = pool.tile([P, TOT1 * 8], i16)
            nc.sync.dma_start(out=idx_t[:], in_=idxs[:])
            hr_t = pool.tile([P, TOT1], f32)
            nc.sync.dma_start(out=hr_t[:], in_=herel[:])
            io_t = pool.tile([P, P], f32)
            nc.sync.dma_start(out=io_t[:], in_=iota[:])

            for b in range(NB1):
                tot = int(cbm[b].sum())
                ps = pp.tile([P, D + 1], f32, tag="ps1", space="PSUM")
                ci = 0
                for k in range(NK):
                    nch_all = int(cbm[b, k])
                    j0 = 0
                    while j0 < nch_all:
                        nch = min(MAXCH, nch_all - j0)
                        base = col_of[(b, k, j0)]
                        dst = gpool.tile([P, nch, EW], f32, tag="gdst")
                        nidx = nch * P
                        nc.gpsimd.dma_gather(
                            dst[:], gk[k][:],
                            idx_t[:, base * 8 : (base + nch) * 8],
                            nidx, nidx, EW,
                        )
                        for j in range(nch):
                            col = base + j
                            sel = wpool.tile([P, P], f32, tag="sel")
                            nc.vector.tensor_tensor(
                                out=sel[:],
                                in0=hr_t[:, col : col + 1].to_broadcast([P, P]),
                                in1=io_t[:],
                                op=mybir.AluOpType.is_equal,
                            )
                            nc.tensor.matmul(
                                out=ps[:], lhsT=sel[:], rhs=dst[:, j, : D + 1],
                                start=(ci == 0), stop=(ci == tot - 1),
                            )
                            ci += 1
                        j0 += nch
                res = wpool.tile([P, D + 1], f32, tag="res1")
                nc.vector.tensor_copy(out=res[:], in_=ps[:])
                nrow = min(P, HSH - b * P)
                nc.sync.dma_start(
                    out=he_part[b * P : b * P + nrow, :], in_=res[:nrow, :]
                )
    return nc


def _build_launch_c(cb2, col_of2):
    TOT2 = int(cb2.sum())
    NW = (N + P - 1) // P  # 782
    nc = bacc.Bacc("TRN2")
    hew = nc.declare_dram_parameter("hew", [P, HT, D + 1], f32, isOutput=False)
    exan_w = nc.declare_dram_parameter("exan_w", [P, NW], f32, isOutput=False)
    cnt_w = nc.declare_dram_parameter("cnt_w", [P, NW], f32, isOutput=False)
    exan_sh = nc.declare_dram_parameter("exan_sh", [P, NB2], f32, isOutput=False)
    ae_bc = nc.declare_dram_parameter("ae_bc", [P, D], f32, isOutput=False)
    iota = nc.declare_dram_parameter("iota", [P, P], f32, isOutput=False)
    ones_col = nc.declare_dram_parameter("ones_col", [P, 1], f32, isOutput=False)
    ones_row = nc.declare_dram_parameter("ones_row", [1, P], f32, isOutput=False)
    idxs = nc.declare_dram_parameter("idxs", [P, TOT2 * 8], i16, isOutput=False)
    norel = nc.declare_dram_parameter("norel", [P, TOT2], f32, isOutput=False)
    h_n = nc.declare_dram_parameter("h_n", [NSH, D], f32, isOutput=True)

    NG2 = (NB2 + GB2 - 1) // GB2
    cbl = cb2.tolist()
    with tile.TileContext(nc) as tc:
        with (
            tc.tile_pool(name="sbuf", bufs=1) as pool,
            tc.tile_pool(name="gpool", bufs=4) as gpool,
            tc.tile_pool(name="work", bufs=4) as wpool,
            tc.tile_pool(name="psum", bufs=3, space="PSUM") as pp,
            tc.tile_pool(name="pscl", bufs=1, space="PSUM") as pscl,
            tc.tile_pool(name="dram", bufs=1, space="DRAM") as dpool,
        ):
            het = pool.tile([P, HT, D + 1], f32)
            nc.sync.dma_start(out=het[:], in_=hew[:])
            exw = pool.tile([P, NW], f32)
            nc.sync.dma_start(out=exw[:], in_=exan_w[:])
            cw = pool.tile([P, NW], f32)
            nc.sync.dma_start(out=cw[:], in_=cnt_w[:])
            exsh = pool.tile([P, NB2], f32)
            nc.sync.dma_start(out=exsh[:], in_=exan_sh[:])
            aet = pool.tile([P, D], f32)
            nc.sync.dma_start(out=aet[:], in_=ae_bc[:])
            io_t = pool.tile([P, P], f32)
            nc.sync.dma_start(out=io_t[:], in_=iota[:])
            onc = pool.tile([P, 1], f32)
            nc.sync.dma_start(out=onc[:], in_=ones_col[:])
            onr = pool.tile([1, P], f32)
            nc.sync.dma_start(out=onr[:], in_=ones_row[:])
            idx_t = pool.tile([P, TOT2 * 8], i16)
            nc.sync.dma_start(out=idx_t[:], in_=idxs[:])
            nr_t = pool.tile([P, TOT2], f32)
            nc.sync.dma_start(out=nr_t[:], in_=norel[:])

            # ---- S1 = sum(counts * exan) ----
            t1 = pool.tile([P, NW], f32)
            r1 = pool.tile([P, 1], f32)
            nc.vector.tensor_tensor(
                out=t1[:], in0=exw[:], in1=cw[:], op=mybir.AluOpType.mult
            )
            nc.vector.tensor_reduce(
                out=r1[:], in_=t1[:], axis=mybir.AxisListType.X,
                op=mybir.AluOpType.add,
            )
            s1p = pscl.tile([1, 1], f32, tag="s1p", space="PSUM")
            nc.tensor.matmul(out=s1p[:], lhsT=r1[:], rhs=onc[:], start=True, stop=True)
            rs1 = pool.tile([1, 1], f32)
            nc.vector.reciprocal(out=rs1[:], in_=s1p[:])
            rs1b = pscl.tile([P, 1], f32, tag="rs1b", space="PSUM")
            nc.tensor.matmul(out=rs1b[:], lhsT=onr[:], rhs=rs1[:], start=True, stop=True)
            rs1c = pool.tile([P, 1], f32)
            nc.vector.tensor_copy(out=rs1c[:], in_=rs1b[:])

            # ---- T2 table + S2 ----
            t2d = dpool.tile([HP, D], f32)
            s2acc = pool.tile([P, 1], f32)
            nc.vector.memset(s2acc[:], 0)
            for t in range(HT):
                ttile = het[:, t, :]
                tmp = wpool.tile([P, D], f32, tag="tmp2")
                araw = wpool.tile([P, 1], f32, tag="araw")
                nc.vector.tensor_tensor(
                    out=tmp[:], in0=ttile[:, 0:D], in1=aet[:],
                    op=mybir.AluOpType.mult,
                )
                nc.vector.tensor_reduce(
                    out=araw[:], in_=tmp[:], axis=mybir.AxisListType.X,
                    op=mybir.AluOpType.add,
                )
                ucol = wpool.tile([P, 1], f32, tag="ucol")
                nc.scalar.activation(
                    out=ucol[:], in_=araw[:],
                    func=mybir.ActivationFunctionType.Exp, scale=rs1c[:],
                )
                wcol = wpool.tile([P, 1], f32, tag="wcol")
                nc.vector.tensor_tensor(
                    out=wcol[:], in0=ucol[:], in1=rs1c[:], op=mybir.AluOpType.mult
                )
                t2t = wpool.tile([P, D], f32, tag="t2t")
                nc.scalar.activation(
                    out=t2t[:], in_=ttile[:, 0:D],
                    func=mybir.ActivationFunctionType.Copy, scale=wcol[:],
                )
                nc.sync.dma_start(out=t2d[t * P : (t + 1) * P, :], in_=t2t[:])
                s2c = wpool.tile([P, 1], f32, tag="s2c")
                nc.vector.tensor_tensor(
                    out=s2c[:], in0=ucol[:], in1=ttile[:, D : D + 1],
                    op=mybir.AluOpType.mult,
                )
                nc.vector.tensor_tensor(
                    out=s2acc[:], in0=s2acc[:], in1=s2c[:], op=mybir.AluOpType.add
                )
            s2p = pscl.tile([1, 1], f32, tag="s2p", space="PSUM")
            nc.tensor.matmul(out=s2p[:], lhsT=s2acc[:], rhs=onc[:], start=True, stop=True)
            rs2 = pool.tile([1, 1], f32)
            nc.vector.reciprocal(out=rs2[:], in_=s2p[:])
            rs2b = pscl.tile([P, 1], f32, tag="rs2b", space="PSUM")
            nc.tensor.matmul(out=rs2b[:], lhsT=onr[:], rhs=rs2[:], start=True, stop=True)
            rs2c = pool.tile([P, 1], f32)
            nc.vector.tensor_copy(out=rs2c[:], in_=rs2b[:])
            vs2 = pool.tile([P, NB2], f32)
            nc.vector.tensor_tensor(
                out=vs2[:], in0=exsh[:], in1=rs2c[:].to_broadcast([P, NB2]),
                op=mybir.AluOpType.mult,
            )

            # ---- stage 2 ----
            for b in range(NB2):
                tot = cbl[b]
                ps = pp.tile([P, D], f32, tag="ps2", space="PSUM")
                ci = 0
                j0 = 0
                while j0 < tot:
                    nch = min(MAXCH, tot - j0)
                    base = col_of2[(b, j0)]
                    dst = gpool.tile([P, nch, D], f32, tag="g2")
                    nidx = nch * P
                    nc.gpsimd.dma_gather(
                        dst[:], t2d[:], idx_t[:, base * 8 : (base + nch) * 8],
                        nidx, nidx, D,
                    )
                    for j in range(nch):
                        col = base + j
                        sel = wpool.tile([P, P], f32, tag="sel2")
                        nc.vector.tensor_tensor(
                            out=sel[:],
                            in0=nr_t[:, col : col + 1].to_broadcast([P, P]),
                            in1=io_t[:],
                            op=mybir.AluOpType.is_equal,
                        )
                        nc.tensor.matmul(
                            out=ps[:], lhsT=sel[:], rhs=dst[:, j, :],
                            start=(ci == 0), stop=(ci == tot - 1),
                        )
                        ci += 1
                    j0 += nch
                hn_t = wpool.tile([P, D], f32, tag="hn")
                nc.vector.tensor_tensor(
                    out=hn_t[:], in0=ps[:],
                    in1=vs2[:, b : b + 1].to_broadcast([P, D]),
                    op=mybir.AluOpType.mult,
                )
                nrow = min(P, NSH - b * P)
                nc.sync.dma_start(
                    out=h_n[b * P : b * P + nrow, :], in_=hn_t[:nrow, :]
                )
    return nc


def kernel(x, W, b, attn_node, attn_edge, node_idx, he_idx, num_hyperedges):
    x = np.asarray(x, np.float32)
    W = np.asarray(W, np.float32)
    b = np.asarray(b, np.float32)
    attn_node = np.asarray(attn_node, np.float32).reshape(-1)
    attn_edge = np.asarray(attn_edge, np.float32).reshape(-1)
    node_idx = np.asarray(node_idx).astype(np.int64)
    he_idx = np.asarray(he_idx).astype(np.int64)
    assert x.shape == (N, D) and node_idx.shape == (M,) and int(num_hyperedges) == H
    LAST_EXEC_TIMES.clear()

    iota_np = np.tile(np.arange(P, dtype=np.float32), (P, 1))

    # ---------- launch A ----------
    nc_a = _build_launch_a()
    xT = np.ascontiguousarray(x.T)  # [128, N]
    b_bc = np.tile(b.reshape(1, D), (P, 1)).astype(np.float32)
    an_bc = np.tile(attn_node.reshape(1, D), (P, 1)).astype(np.float32)
    ins_a = []
    for c in range(NC):
        xts = np.zeros((P, NSHP), np.float32)
        xts[:, :NSH] = xT[:, c * NSH : (c + 1) * NSH]
        ins_a.append({"xT": xts, "W": W, "b_bc": b_bc, "an_bc": an_bc})
    res_a = _run(nc_a, ins_a, "A")
    g_full = np.concatenate([res_a[c]["g_sh"][:NSH] for c in range(NC)], axis=0)
    exan_full = g_full[:, D].copy()  # exp(a_n), [N]

    # ---------- stage-1 host prep ----------
    c1 = he_idx // HSH
    b1 = (he_idx % HSH) // P
    k1 = node_idx // CHUNK
    key1 = ((c1 * NB1 + b1) * NK + k1).astype(np.int64)
    gidx1 = (node_idx - k1 * CHUNK).astype(np.int16)
    rel1 = (he_idx % HSH - b1 * P).astype(np.float32)
    packed1, cb1 = _pack_groups(key1, NB1 * NK, gidx1, rel1, NC, NB1 * NK)
    cb1m = cb1.reshape(NB1, NK)
    # packed order is already (b major, k minor, j) == kernel emission order
    slot_start = np.zeros(NB1 * NK + 1, np.int64)
    np.cumsum(cb1, out=slot_start[1:])
    col_of = {}
    for bb in range(NB1):
        for k in range(NK):
            for j in range(int(cb1m[bb, k])):
                col_of[(bb, k, j)] = int(slot_start[bb * NK + k]) + j
    gks = {f"g{k}": g_full[k * CHUNK : (k + 1) * CHUNK] for k in range(NK)}
    ins_b = []
    for c in range(NC):
        gi, rl = packed1[c]
        ins_b.append(
            {
                **gks,
                "idxs": _wrap16(gi),
                "herel": np.ascontiguousarray(rl.reshape(-1, P).T),
                "iota": iota_np,
            }
        )
    nc_b = _build_launch_b(cb1, col_of)
    res_b = _run(nc_b, ins_b, "B")
    he_part = np.concatenate([res_b[c]["he_part"] for c in range(NC)], axis=0)

    # ---------- stage-2 host prep ----------
    c2 = node_idx // NSH
    b2 = (node_idx % NSH) // P
    key2 = (c2 * NB2 + b2).astype(np.int64)
    gidx2 = he_idx.astype(np.int16)
    rel2 = (node_idx % NSH - b2 * P).astype(np.float32)
    packed2, cb2 = _pack_groups(key2, NB2, gidx2, rel2, NC, NB2)
    col_of2 = {}
    off = 0
    for bb in range(NB2):
        for j in range(int(cb2[bb])):
            col_of2[(bb, j)] = off
            off += 1
    TOT2 = off

    he_pad = np.zeros((HP, D + 1), np.float32)
    he_pad[:H] = he_part
    hew = np.ascontiguousarray(
        he_pad.reshape(HT, P, D + 1).transpose(1, 0, 2)
    )
    NW = (N + P - 1) // P
    exan_p = np.zeros(NW * P, np.float32)
    exan_p[:N] = exan_full
    exan_w = np.ascontiguousarray(exan_p.reshape(NW, P).T)
    cnt = np.bincount(node_idx, minlength=N).astype(np.float32)
    cnt_p = np.zeros(NW * P, np.float32)
    cnt_p[:N] = cnt
    cnt_w = np.ascontiguousarray(cnt_p.reshape(NW, P).T)
    ae_bc = np.tile(attn_edge.reshape(1, D), (P, 1)).astype(np.float32)
    ones_col = np.ones((P, 1), np.float32)
    ones_row = np.ones((1, P), np.float32)

    ins_c = []
    for c in range(NC):
        gi, rl = packed2[c]
        exs_p = np.zeros(NSHP, np.float32)
        exs_p[:NSH] = exan_full[c * NSH : (c + 1) * NSH]
        ins_c.append(
            {
                "hew": hew,
                "exan_w": exan_w,
                "cnt_w": cnt_w,
                "exan_sh": np.ascontiguousarray(exs_p.reshape(NB2, P).T),
                "ae_bc": ae_bc,
                "iota": iota_np,
                "ones_col": ones_col,
                "ones_row": ones_row,
                "idxs": _wrap16(gi),
                "norel": np.ascontiguousarray(rl.reshape(-1, P).T),
            }
        )
    nc_c = _build_launch_c(cb2, col_of2)
    res_c = _run(nc_c, ins_c, "C")
    h_n = np.concatenate([res_c[c]["h_n"] for c in range(NC)], axis=0)
    return h_n

